# revision 2
# baseline (speedup 1.0000x reference)
"""Trainium2 Bass kernel for nn_DigitConvolutionalModel (3x3 valid conv + 3-layer MLP).

Strategy
--------
The 3x3 "valid" cross-correlation is linear in x, so it is folded on the host
into the first MLP weight:  conv(x).reshape(B, 676) @ w1  ==  x @ weff  with
weff[784, 256] built from conv_w and w1.  The device then runs a pure 3-layer
MLP:

    out = relu(relu(x @ weff + b1) @ w2 + b2) @ w3 + b3

Data-parallel over the batch across 8 NeuronCores (8192 rows per core).
On-chip dataflow is feature-major ([features, batch] tiles) so the contraction
dim of every matmul lands on SBUF partitions with zero on-chip transposes; the
host pre-transposes each x shard (and transposes the [10, batch] result back).
Matmuls run as float32r (full-rate fp32 mode at free-dim >= 256).
"""

import numpy as np

import concourse.bass as bass
import concourse.mybir as mybir
import concourse.tile as tile
from concourse import bacc
from concourse.bass_utils import run_bass_kernel_spmd

N_CORES = 8
B = 65536
BS = B // N_CORES          # 8192 batch rows per core
KIN = 784                  # input features (28*28)
KC, KCH = 7, 112           # layer-1 contraction chunks: 7 x 112 = 784
H1, H2, NOUT = 256, 128, 10
NB = 512                   # batch tile (matmul free dim = one PSUM bank of fp32)
NITER = BS // NB           # 16

F32 = mybir.dt.float32
F32R = mybir.dt.float32r
RELU = mybir.ActivationFunctionType.Relu
IDENT = mybir.ActivationFunctionType.Identity


def build_program():
    nc = bacc.Bacc(
        "TRN2", target_bir_lowering=False, debug=False, num_devices=N_CORES
    )
    xT = nc.dram_tensor("xT", [KIN, BS], F32R, kind="ExternalInput").ap()
    weff = nc.dram_tensor("weff", [KIN, H1], F32R, kind="ExternalInput").ap()
    w2 = nc.dram_tensor("w2", [H1, H2], F32R, kind="ExternalInput").ap()
    w3 = nc.dram_tensor("w3", [H2, NOUT], F32R, kind="ExternalInput").ap()
    b1 = nc.dram_tensor("b1", [128, 2], F32, kind="ExternalInput").ap()
    b2 = nc.dram_tensor("b2", [128, 1], F32, kind="ExternalInput").ap()
    b3 = nc.dram_tensor("b3", [NOUT, 1], F32, kind="ExternalInput").ap()
    outT = nc.dram_tensor("outT", [NOUT, BS], F32, kind="ExternalOutput").ap()

    with tile.TileContext(nc) as tc:
        with (
            tc.tile_pool(name="w", bufs=1) as wp,
            tc.tile_pool(name="x", bufs=3) as xp,
            tc.tile_pool(name="h", bufs=2) as hp,
            tc.tile_pool(name="o", bufs=2) as op,
            tc.tile_pool(name="ps", bufs=2, space=bass.MemorySpace.PSUM) as pp,
        ):
            weff_t = []
            for k in range(KC):
                t = wp.tile([KCH, H1], F32R, tag=f"weff{k}")
                nc.sync.dma_start(t[:], weff[k * KCH:(k + 1) * KCH, :])
                weff_t.append(t)
            w2_t = []
            for k in range(2):
                t = wp.tile([128, H2], F32R, tag=f"w2_{k}")
                nc.sync.dma_start(t[:], w2[k * 128:(k + 1) * 128, :])
                w2_t.append(t)
            w3_t = wp.tile([H2, NOUT], F32R, tag="w3")
            nc.sync.dma_start(w3_t[:], w3[:])
            b1_t = wp.tile([128, 2], F32, tag="b1")
            nc.sync.dma_start(b1_t[:], b1[:])
            b2_t = wp.tile([128, 1], F32, tag="b2")
            nc.sync.dma_start(b2_t[:], b2[:])
            b3_t = wp.tile([NOUT, 1], F32, tag="b3")
            nc.sync.dma_start(b3_t[:], b3[:])

            for n in range(NITER):
                nsl = bass.ts(n, NB)
                xs = []
                for k in range(KC):
                    t = xp.tile([KCH, NB], F32R, tag=f"x{k}")
                    nc.sync.dma_start(t[:], xT[k * KCH:(k + 1) * KCH, nsl])
                    xs.append(t)
                h1s = []
                for m in range(2):
                    p1 = pp.tile([128, NB], F32, tag=f"p1_{m}")
                    for k in range(KC):
                        nc.tensor.matmul(
                            p1[:],
                            weff_t[k][:, m * 128:(m + 1) * 128],
                            xs[k][:],
                            start=(k == 0),
                            stop=(k == KC - 1),
                        )
                    h1 = hp.tile([128, NB], F32R, tag=f"h1_{m}")
                    nc.scalar.activation(h1[:], p1[:], RELU, bias=b1_t[:, m:m + 1])
                    h1s.append(h1)
                p2 = pp.tile([128, NB], F32, tag="p2")
                for k in range(2):
                    nc.tensor.matmul(
                        p2[:],
                        w2_t[k][:],
                        h1s[k][:],
                        start=(k == 0),
                        stop=(k == 1),
                    )
                h2 = hp.tile([128, NB], F32R, tag="h2")
                nc.scalar.activation(h2[:], p2[:], RELU, bias=b2_t[:, 0:1])
                p3 = pp.tile([NOUT, NB], F32, tag="p3")
                nc.tensor.matmul(
                    p3[:],
                    w3_t[:],
                    h2[:],
                    start=True,
                    stop=True,
                )
                ot = op.tile([NOUT, NB], F32, tag="ot")
                nc.scalar.activation(ot[:], p3[:], IDENT, bias=b3_t[:, 0:1])
                nc.sync.dma_start(outT[:, nsl], ot[:])

    nc.compile()
    return nc


_NC = None


def _get_program():
    global _NC
    if _NC is None:
        _NC = build_program()
    return _NC


def make_in_maps(x, conv_w, w1, b1, w2, b2, w3, b3):
    """Host-side prep: fold conv into w1, shard + transpose x, reshape biases."""
    conv_w = np.asarray(conv_w, np.float64)
    w1r = np.asarray(w1, np.float64).reshape(26, 26, H1)
    weff = np.zeros((28, 28, H1), np.float64)
    for u in range(3):
        for v in range(3):
            weff[u:u + 26, v:v + 26, :] += conv_w[u, v] * w1r
    weff = np.ascontiguousarray(weff.reshape(KIN, H1).astype(np.float32))

    b1d = np.ascontiguousarray(np.asarray(b1, np.float32).reshape(2, 128).T)
    b2d = np.ascontiguousarray(np.asarray(b2, np.float32).reshape(128, 1))
    b3d = np.ascontiguousarray(np.asarray(b3, np.float32).reshape(NOUT, 1))
    w2c = np.ascontiguousarray(np.asarray(w2, np.float32))
    w3c = np.ascontiguousarray(np.asarray(w3, np.float32))

    x = np.asarray(x, np.float32)
    in_maps = []
    for c in range(N_CORES):
        xs = np.ascontiguousarray(x[c * BS:(c + 1) * BS].T)
        in_maps.append({
            "xT": xs, "weff": weff, "w2": w2c, "w3": w3c,
            "b1": b1d, "b2": b2d, "b3": b3d,
        })
    return in_maps


def run(x, conv_w, w1, b1, w2, b2, w3, b3, trace=False):
    nc = _get_program()
    in_maps = make_in_maps(x, conv_w, w1, b1, w2, b2, w3, b3)
    br = run_bass_kernel_spmd(nc, in_maps, core_ids=list(range(N_CORES)),
                              trace=trace)
    out = np.empty((B, NOUT), np.float32)
    for c in range(N_CORES):
        out[c * BS:(c + 1) * BS] = br.results[c]["outT"].T
    return out, br


def kernel(x, conv_w, w1, b1, w2, b2, w3, b3):
    out, _ = run(x, conv_w, w1, b1, w2, b2, w3, b3)
    return out


# revision 3
# speedup vs baseline: 1.1831x; 1.1831x over previous
"""Trainium2 Bass kernel for nn_DigitConvolutionalModel (3x3 valid conv + 3-layer MLP).

Strategy
--------
The 3x3 "valid" cross-correlation is linear in x, so it is folded on the host
into the first MLP weight:  conv(x).reshape(B, 676) @ w1  ==  x @ weff  with
weff[784, 256] built from conv_w and w1.  The device then runs a pure 3-layer
MLP:

    out = relu(relu(x @ weff + b1) @ w2 + b2) @ w3 + b3

Data-parallel over the batch across 8 NeuronCores (8192 rows per core).
On-chip dataflow is feature-major ([features, batch] tiles) so the contraction
dim of every matmul lands on SBUF partitions with zero on-chip transposes; the
host pre-transposes each x shard (and transposes the [10, batch] result back).
Matmuls run as float32r (full-rate fp32 mode at free-dim >= 256).
"""

import numpy as np

import concourse.bass as bass
import concourse.mybir as mybir
import concourse.tile as tile
from concourse import bacc
from concourse.bass_utils import run_bass_kernel_spmd

N_CORES = 8
B = 65536
BS = B // N_CORES          # 8192 batch rows per core
KIN = 784                  # input features (28*28)
KC, KCH = 7, 112           # layer-1 contraction chunks: 7 x 112 = 784
H1, H2, NOUT = 256, 128, 10
NB = 512                   # batch tile (matmul free dim = one PSUM bank of fp32)
NITER = BS // NB           # 16

F32 = mybir.dt.float32
F32R = mybir.dt.float32r
RELU = mybir.ActivationFunctionType.Relu
IDENT = mybir.ActivationFunctionType.Identity


def build_program():
    nc = bacc.Bacc(
        "TRN2", target_bir_lowering=False, debug=False, num_devices=N_CORES
    )
    xT = nc.dram_tensor("xT", [KIN, BS], F32R, kind="ExternalInput").ap()
    weff = nc.dram_tensor("weff", [KIN, H1], F32R, kind="ExternalInput").ap()
    w2 = nc.dram_tensor("w2", [H1, H2], F32R, kind="ExternalInput").ap()
    w3 = nc.dram_tensor("w3", [H2, NOUT], F32R, kind="ExternalInput").ap()
    b1 = nc.dram_tensor("b1", [128, 2], F32, kind="ExternalInput").ap()
    b2 = nc.dram_tensor("b2", [128, 1], F32, kind="ExternalInput").ap()
    b3 = nc.dram_tensor("b3", [NOUT, 1], F32, kind="ExternalInput").ap()
    outT = nc.dram_tensor("outT", [NOUT, BS], F32, kind="ExternalOutput").ap()

    # feature-chunked views: (k p) j -> p k j puts the 112-row feature chunk
    # on partitions so one 3D DMA moves all 7 chunks of a batch tile
    xTv = xT.rearrange("(k p) j -> p k j", k=KC)
    weffv = weff.rearrange("(k p) j -> p k j", k=KC)
    w2v = w2.rearrange("(k p) j -> p k j", k=2)

    with tile.TileContext(nc) as tc:
        with (
            tc.tile_pool(name="w", bufs=1) as wp,
            tc.tile_pool(name="x", bufs=4) as xp,
            tc.tile_pool(name="h", bufs=2) as hp,
            tc.tile_pool(name="o", bufs=2) as op,
            tc.tile_pool(name="ps", bufs=2, space=bass.MemorySpace.PSUM) as pp,
        ):
            weff_t = wp.tile([KCH, KC, H1], F32R, tag="weff")
            nc.sync.dma_start(weff_t[:], weffv[:])
            w2_t = wp.tile([128, 2, H2], F32R, tag="w2")
            nc.sync.dma_start(w2_t[:], w2v[:])
            w3_t = wp.tile([H2, NOUT], F32R, tag="w3")
            nc.sync.dma_start(w3_t[:], w3[:])
            b1_t = wp.tile([128, 2], F32, tag="b1")
            nc.sync.dma_start(b1_t[:], b1[:])
            b2_t = wp.tile([128, 1], F32, tag="b2")
            nc.sync.dma_start(b2_t[:], b2[:])
            b3_t = wp.tile([NOUT, 1], F32, tag="b3")
            nc.sync.dma_start(b3_t[:], b3[:])

            for n in range(NITER):
                nsl = bass.ts(n, NB)
                xt = xp.tile([KCH, KC, NB], F32R, tag="x")
                nc.sync.dma_start(xt[:], xTv[:, :, nsl])
                h1s = []
                for m in range(2):
                    p1 = pp.tile([128, NB], F32, tag=f"p1_{m}")
                    for k in range(KC):
                        nc.tensor.matmul(
                            p1[:],
                            weff_t[:, k, m * 128:(m + 1) * 128],
                            xt[:, k, :],
                            start=(k == 0),
                            stop=(k == KC - 1),
                        )
                    h1 = hp.tile([128, NB], F32R, tag=f"h1_{m}")
                    nc.scalar.activation(h1[:], p1[:], RELU, bias=b1_t[:, m:m + 1])
                    h1s.append(h1)
                p2 = pp.tile([128, NB], F32, tag="p2")
                for k in range(2):
                    nc.tensor.matmul(
                        p2[:],
                        w2_t[:, k, :],
                        h1s[k][:],
                        start=(k == 0),
                        stop=(k == 1),
                    )
                h2 = hp.tile([128, NB], F32R, tag="h2")
                nc.vector.tensor_scalar(
                    h2[:], p2[:], b2_t[:, 0:1], 0.0,
                    mybir.AluOpType.add, mybir.AluOpType.max,
                )
                p3 = pp.tile([NOUT, NB], F32, tag="p3")
                nc.tensor.matmul(
                    p3[:],
                    w3_t[:],
                    h2[:],
                    start=True,
                    stop=True,
                )
                ot = op.tile([NOUT, NB], F32, tag="ot")
                nc.vector.tensor_scalar(
                    ot[:], p3[:], b3_t[:, 0:1], None, mybir.AluOpType.add,
                )
                nc.gpsimd.dma_start(outT[:, nsl], ot[:])

    nc.compile()
    return nc


_NC = None


def _get_program():
    global _NC
    if _NC is None:
        _NC = build_program()
    return _NC


def make_in_maps(x, conv_w, w1, b1, w2, b2, w3, b3):
    """Host-side prep: fold conv into w1, shard + transpose x, reshape biases."""
    conv_w = np.asarray(conv_w, np.float64)
    w1r = np.asarray(w1, np.float64).reshape(26, 26, H1)
    weff = np.zeros((28, 28, H1), np.float64)
    for u in range(3):
        for v in range(3):
            weff[u:u + 26, v:v + 26, :] += conv_w[u, v] * w1r
    weff = np.ascontiguousarray(weff.reshape(KIN, H1).astype(np.float32))

    b1d = np.ascontiguousarray(np.asarray(b1, np.float32).reshape(2, 128).T)
    b2d = np.ascontiguousarray(np.asarray(b2, np.float32).reshape(128, 1))
    b3d = np.ascontiguousarray(np.asarray(b3, np.float32).reshape(NOUT, 1))
    w2c = np.ascontiguousarray(np.asarray(w2, np.float32))
    w3c = np.ascontiguousarray(np.asarray(w3, np.float32))

    x = np.asarray(x, np.float32)
    in_maps = []
    for c in range(N_CORES):
        xs = np.ascontiguousarray(x[c * BS:(c + 1) * BS].T)
        in_maps.append({
            "xT": xs, "weff": weff, "w2": w2c, "w3": w3c,
            "b1": b1d, "b2": b2d, "b3": b3d,
        })
    return in_maps


def run(x, conv_w, w1, b1, w2, b2, w3, b3, trace=False):
    nc = _get_program()
    in_maps = make_in_maps(x, conv_w, w1, b1, w2, b2, w3, b3)
    br = run_bass_kernel_spmd(nc, in_maps, core_ids=list(range(N_CORES)),
                              trace=trace)
    out = np.empty((B, NOUT), np.float32)
    for c in range(N_CORES):
        out[c * BS:(c + 1) * BS] = br.results[c]["outT"].T
    return out, br


def kernel(x, conv_w, w1, b1, w2, b2, w3, b3):
    out, _ = run(x, conv_w, w1, b1, w2, b2, w3, b3)
    return out
